# revision 42
# baseline (speedup 1.0000x reference)
"""Multi-head graph attention (GAT) kernel for 8 Trainium2 NeuronCores.

Math (per batch b, head h):
  Wh = h @ W_h                        [N, HD]
  si = Wh @ a1_h ; sj = Wh @ a2_h     [N]
  e[n, m] = leaky_relu(si[n] + sj[m], 0.2), masked where adj[n, m] == 0
  alpha = softmax(e, axis=-1); out = alpha @ Wh; concat heads; proj; +h; LN

Device identity chain (per softmax row n; factors in si[n] cancel):
  exp(leaky(y)) = exp(0.2*y + 0.8*relu(y))
               ~= exp(0.8*relu(y) + 0.2*sj[m])           (mod exp(0.2 si[n]))
Scores are built transposed E^T[m, n] (m on partitions) so tiles feed the
attention*V matmul directly, with a ones column in the stationary [Wh | 1]
producing softmax row-sums for free.

Most tiles evaluate the exp as a Schraudolph bit-trick in ONE fused
tensor_scalar (max distributes over the affine map; A = 2^7/ln2):
  bits = max(0.8*A*si[n] + (A*sj[m] + B'), (0.2*A*sj[m] + B'))   -> int16
bitcast(int16 -> bf16) is exp(0.8*relu(y) + 0.2*sj) to ~2% relative.
One pair per head instead runs exact Relu+Exp on the Activation engine to
balance engine load. Masking multiplies by the adjacency plane (bf16),
split between DVE and GPSIMD.

Sharding: batch b -> core b (B == 8 == n_cores). adj/params replicated.
"""

import os
import sys

for _p in ("/opt/trn_rl_repo", "/root/.axon_site/_ro/trn_rl_repo"):
    if os.path.isdir(_p) and _p not in sys.path:
        sys.path.insert(0, _p)

import numpy as np
import ml_dtypes

import concourse.bass as bass
import concourse.bacc as bacc
import concourse.tile as tile
import concourse.mybir as mybir
from concourse.bass import ts
from concourse.bass_utils import run_bass_kernel_spmd

B, N, D, H, HD = 8, 1024, 256, 4, 64
P = 128
NCH = N // P  # 8 chunks of the node axis
KCH = D // P  # 2 chunks of the feature axis
EPS = 1e-5

F32 = mybir.dt.float32
BF16 = mybir.dt.bfloat16
I16 = mybir.dt.int16
I32 = mybir.dt.int32

A16 = 128.0 / np.log(2.0)         # Schraudolph scale for bf16 bit layout
BOFF = 127.0 * 128.0 - 5.7        # exponent bias + error-centering offset
WCOLS = H * HD + H + H            # [Wcat | csj | csi'] columns

_CACHE = {}


def _build_bass():
    nc = bacc.Bacc("TRN2", target_bir_lowering=False, debug=False)

    h_d = nc.dram_tensor("h_b", [N, D], BF16, kind="ExternalInput").ap()
    hT_d = nc.dram_tensor("hT_b", [D, N], BF16, kind="ExternalInput").ap()
    adjt_d = nc.dram_tensor("adjT", [N, N], BF16, kind="ExternalInput").ap()
    liq_d = nc.dram_tensor("liq", [N, N], I16, kind="ExternalInput").ap()
    wcc_d = nc.dram_tensor("wcc", [D, WCOLS], BF16, kind="ExternalInput").ap()
    pwt_d = nc.dram_tensor("pwT", [D, D], BF16, kind="ExternalInput").ap()
    pb_d = nc.dram_tensor("pb", [1, D], BF16, kind="ExternalInput").ap()
    gam_d = nc.dram_tensor("gamma", [1, D], F32, kind="ExternalInput").ap()
    bet_d = nc.dram_tensor("beta", [1, D], F32, kind="ExternalInput").ap()
    out_d = nc.dram_tensor("out_b", [N, D], F32, kind="ExternalOutput").ap()
    sis_d = nc.dram_tensor("si_scr", [H, N], BF16, kind="Internal").ap()
    rrs_d = nc.dram_tensor("rr_scr", [H, N], BF16, kind="Internal").ap()

    with tile.TileContext(nc) as tc:
        _emit(nc, tc, h_d, hT_d, adjt_d, liq_d, wcc_d, pwt_d, pb_d, gam_d,
              bet_d, out_d, sis_d, rrs_d)
    nc.compile()
    return nc


def _emit(nc, tc, h_d, hT_d, adjt_d, liq_d, wcc_d, pwt_d, pb_d, gam_d,
          bet_d, out_d, sis_d, rrs_d):
    import contextlib

    add = mybir.AluOpType.add
    mult = mybir.AluOpType.mult
    amax = mybir.AluOpType.max
    ashr = mybir.AluOpType.arith_shift_right
    Copy = mybir.ActivationFunctionType.Copy
    Ident = mybir.ActivationFunctionType.Identity
    Relu = mybir.ActivationFunctionType.Relu
    Exp = mybir.ActivationFunctionType.Exp

    ctx = contextlib.ExitStack()
    with ctx:
        const = ctx.enter_context(tc.tile_pool(name="const", bufs=1))
        big = ctx.enter_context(tc.tile_pool(name="big", bufs=1))
        work = ctx.enter_context(tc.tile_pool(name="work", bufs=4))
        small = ctx.enter_context(tc.tile_pool(name="small", bufs=1))
        pss = ctx.enter_context(tc.tile_pool(name="pss", bufs=2, space="PSUM"))
        psg = ctx.enter_context(tc.tile_pool(name="psg", bufs=3, space="PSUM"))

        # tiny dummy exp: absorbs the one activation-table load at t=0
        dummy = const.tile([1, 1], BF16)
        nc.vector.memset(dummy, 0.0)
        nc.scalar.activation(out=dummy, in_=dummy, func=Exp)

        # ---- phase-0 loads: only what the si chain needs ------------------
        hT_sb = big.tile([P, KCH, N], BF16)
        hT_r = hT_d.rearrange("(k p) n -> p k n", p=P)
        for k in range(KCH):
            nc.sync.dma_start(out=hT_sb[:, k, :], in_=hT_r[:, k, :])
        wcc_sb = const.tile([P, KCH, WCOLS], BF16)
        nc.sync.dma_start(out=wcc_sb,
                          in_=wcc_d.rearrange("(k p) m -> p k m", p=P))
        adjt_sb = big.tile([P, NCH, N], BF16)
        adjt_r = adjt_d.rearrange("(c p) n -> p c n", p=P)
        liq_r = liq_d.rearrange("(c p) n -> p c n", p=P)

        # ---- si rows (pre-scaled 0.8*A16) -> DRAM -> per-head broadcast ---
        sirow = small.tile([H, N], BF16, tag="sirow")
        for s_ in range(2):
            ps_si = pss.tile([H, 512], F32, tag="ps")
            for k in range(KCH):
                nc.tensor.matmul(
                    ps_si, lhsT=wcc_sb[:, k, H * HD + H:],
                    rhs=hT_sb[:, k, ts(s_, 512)],
                    start=(k == 0), stop=(k == KCH - 1))
            nc.scalar.activation(out=sirow[:, ts(s_, 512)], in_=ps_si,
                                 func=Copy)
        nc.sync.dma_start(out=sis_d, in_=sirow)
        sibca = big.tile([P, H, N], BF16)
        nc.sync.dma_start(
            out=sibca[:, 0:1, :],
            in_=bass.AP(tensor=sis_d.tensor, offset=sis_d.offset,
                        ap=[[0, P], [N, 1], [1, N]]))
        nc.sync.dma_start(
            out=sibca[:, 1:4, :],
            in_=bass.AP(tensor=sis_d.tensor, offset=sis_d.offset + N,
                        ap=[[0, P], [N, 3], [1, N]]))

        # ---- phase-1 loads: everything else -------------------------------
        for c2 in (0, 4):
            nc.sync.dma_start(out=adjt_sb[:, c2:c2 + 2, :],
                              in_=adjt_r[:, c2:c2 + 2, :])
        pwt_sb = const.tile([P, KCH, D], BF16)
        nc.sync.dma_start(out=pwt_sb,
                          in_=pwt_d.rearrange("(k p) m -> p k m", p=P))
        pb_sb = const.tile([1, D], BF16)
        nc.sync.dma_start(out=pb_sb, in_=pb_d)
        h_sb = big.tile([P, NCH, D], BF16)
        nc.sync.dma_start(out=h_sb, in_=h_d.rearrange("(c p) d -> p c d", p=P))
        gam_bc = const.tile([P, D], F32)
        nc.sync.dma_start(
            out=gam_bc, in_=bass.AP(tensor=gam_d.tensor, offset=gam_d.offset,
                                    ap=[[0, P], [1, D]]))
        bet_bc = const.tile([P, D], F32)
        nc.sync.dma_start(
            out=bet_bc, in_=bass.AP(tensor=bet_d.tensor, offset=bet_d.offset,
                                    ap=[[0, P], [1, D]]))

        ones_sb = const.tile([1, P], BF16)
        nc.vector.memset(ones_sb, 1.0)
        ident = const.tile([P, P], BF16)
        from concourse.masks import make_identity
        make_identity(nc, ident)

        whs_sb = big.tile([P, NCH, H, HD + 1], BF16)
        nc.vector.memset(whs_sb[:, :, :, HD:HD + 1], 1.0)

        # ---- Wh + sj columns for all chunks ------------------------------
        s1c = big.tile([P, NCH, H], F32)   # A16*sj + BOFF       (bits dom)
        s2c = big.tile([P, NCH, H], F32)   # 0.2*A16*sj + BOFF   (bits dom)
        for c in range(NCH):
            ps = pss.tile([P, H * HD + H], F32, tag="ps")
            for k in range(KCH):
                nc.tensor.matmul(
                    ps, lhsT=hT_sb[:, k, ts(c, P)],
                    rhs=wcc_sb[:, k, 0:H * HD + H],
                    start=(k == 0), stop=(k == KCH - 1))
            nc.scalar.activation(
                out=whs_sb[:, c, :, 0:HD],
                in_=ps[:, 0:H * HD].rearrange("p (h d) -> p h d", h=H),
                func=Copy)
            nc.vector.tensor_scalar(
                out=s1c[:, c, :], in0=ps[:, H * HD:], scalar1=float(A16),
                scalar2=float(BOFF), op0=mult, op1=add)
            nc.vector.tensor_scalar(
                out=s2c[:, c, :], in0=ps[:, H * HD:], scalar1=float(0.2 * A16),
                scalar2=float(BOFF), op0=mult, op1=add)
        # ACT-domain columns derived from the bits-domain ones:
        #   s3c = 0.8*A16*sj = 0.8*(s1c - BOFF);  s4c = 0.2*sj = (s2c-BOFF)/A16
        s3c = big.tile([P, NCH, H], F32)
        nc.vector.tensor_scalar(
            out=s3c, in0=s1c, scalar1=0.8, scalar2=float(-0.8 * BOFF),
            op0=mult, op1=add)
        s4c = big.tile([P, NCH, H], F32)
        nc.vector.tensor_scalar(
            out=s4c, in0=s2c, scalar1=float(1.0 / A16),
            scalar2=float(-BOFF / A16), op0=mult, op1=add)

        # ---- scores + A@V --------------------------------------------------
        # Mask engines per pair: 1 -> GPSIMD (longest cover, consumed last),
        # 3 -> DMA compute-copy (multiplies adjacency in from DRAM), 0/2 ->
        # DVE. Matmuls consume [2, 3, 0, 1]. Softmax normalization runs at a
        # two-head lag: reciprocal row -> DRAM -> partition-broadcast back,
        # then a PSUM x SBUF multiply drains into hmT.
        hmT = big.tile([P, KCH, N], BF16)
        psg_of = {}
        rr_of = {}
        rrbc_of = {}
        gm_of = {}
        DMA_PAIRS = {(hh, 3) for hh in range(H)}

        act_g_of = {}

        def emit_act_pair(hh):
            gm2 = work.tile([P, 2, N], BF16, tag="gm2a", bufs=3)
            for j in range(2):
                r2 = work.tile([P, N], BF16, tag="r2")
                nc.scalar.activation(
                    out=r2, in_=sibca[:, hh, :], func=Relu,
                    bias=s3c[:, j, hh:hh + 1])
                nc.scalar.activation(
                    out=gm2[:, j, :], in_=r2, func=Exp,
                    bias=s4c[:, j, hh:hh + 1], scale=float(1.0 / A16))
            act_g_of[hh] = gm2

        def emit_gmm(hh):
            gmm = work.tile([P, 2, N], BF16, tag="gmm", bufs=2)
            nc.vector.tensor_tensor(
                out=gmm, in0=act_g_of[hh], in1=adjt_sb[:, 0:2, :], op=mult)
            gm_of[(hh, 0)] = gmm

        def emit_g2(hh, mcp):
            g2 = work.tile([P, 2, N], I16, tag="g2", bufs=8)
            for j in range(2):
                mc = 2 * mcp + j
                nc.vector.tensor_scalar(
                    out=g2[:, j, :], in0=sibca[:, hh, :],
                    scalar1=s1c[:, mc, hh:hh + 1],
                    scalar2=s2c[:, mc, hh:hh + 1], op0=add, op1=amax)
            return g2

        def emit_mask(hh, mcp, g2):
            if (hh, mcp) in DMA_PAIRS:
                # mask via DMA compute-add: bits += {0, -3000} plane (int16)
                nc.gpsimd.dma_start(
                    out=g2, in_=liq_r[:, 2 * mcp:2 * mcp + 2, :],
                    accum_op=add)
                gm_of[(hh, mcp)] = g2.bitcast(BF16)
                return
            gm2 = work.tile([P, 2, N], BF16, tag="gm2", bufs=4)
            eng = nc.gpsimd if mcp == 1 else nc.vector
            if mcp == 1:
                for j in range(2):
                    eng.tensor_tensor(
                        out=gm2[:, j, :], in0=g2[:, j, :].bitcast(BF16),
                        in1=adjt_sb[:, 2 * mcp + j, :], op=mult)
            else:
                eng.tensor_tensor(
                    out=gm2, in0=g2.bitcast(BF16),
                    in1=adjt_sb[:, 2 * mcp:2 * mcp + 2, :], op=mult)
            gm_of[(hh, mcp)] = gm2

        def emit_pairs(hh):
            emit_act_pair(hh)
            g1 = emit_g2(hh, 1)
            emit_mask(hh, 1, g1)
            g3 = emit_g2(hh, 3)
            emit_mask(hh, 3, g3)
            g2_ = emit_g2(hh, 2)
            emit_mask(hh, 2, g2_)
            emit_gmm(hh)

        def emit_mms(hh, mcp, first, last):
            pg = psg_of[hh]
            gm = gm_of[(hh, mcp)]
            for j in range(2):
                for s_ in range(2):
                    nc.tensor.matmul(
                        pg[:, ts(s_, 512)],
                        lhsT=whs_sb[:, 2 * mcp + j, hh, :],
                        rhs=gm[:, j, ts(s_, 512)],
                        start=(first and j == 0), stop=(last and j == 1))

        def emit_recip(hh, bcast=True):
            pg = psg_of[hh]
            rr1 = work.tile([1, N], BF16, tag="rr1")
            rr_of[hh] = rr1
            with nc.allow_low_precision(reason="bf16 softmax scale"):
                nc.vector.reciprocal(out=rr1, in_=pg[HD:HD + 1, :])
            if not bcast:
                return
            nc.sync.dma_start(
                out=bass.AP(tensor=rrs_d.tensor, offset=rrs_d.offset + hh * N,
                            ap=[[1, N]]),
                in_=rr1)
            rrbc = work.tile([64, N], BF16, tag="rrbc")
            rrbc_of[hh] = rrbc
            nc.sync.dma_start(
                out=rrbc,
                in_=bass.AP(tensor=rrs_d.tensor, offset=rrs_d.offset + hh * N,
                            ap=[[0, 64], [1, N]]))

        def emit_norm(hh):
            pg = psg_of[hh]
            pr = 64 * (hh % 2)
            nc.vector.tensor_tensor(
                out=hmT[pr:pr + 64, hh // 2, :], in0=pg[0:HD, :],
                in1=rrbc_of[hh], op=mult)

        def emit_norm_pss(hh):
            # DMA-free tail normalization for the last head
            pg = psg_of[hh]
            rr1 = rr_of[hh]
            hm_un = work.tile([64, N], BF16, tag="hmun")
            nc.scalar.activation(out=hm_un, in_=pg[0:HD, :], func=Copy)
            pr = 64 * (hh % 2)
            for s_ in range(2):
                psrr = pss.tile([64, 512], F32, tag="ps")
                nc.tensor.matmul(psrr, lhsT=ones_sb[0:1, 0:64],
                                 rhs=rr1[0:1, ts(s_, 512)],
                                 start=True, stop=True)
                nc.vector.tensor_tensor(
                    out=hmT[pr:pr + 64, hh // 2, ts(s_, 512)],
                    in0=hm_un[:, ts(s_, 512)], in1=psrr, op=mult)

        for hh in range(H):
            psg_of[hh] = psg.tile([HD + 1, N], F32, tag="av", name=f"pg{hh}")
        emit_pairs(0)
        for hh in range(H):
            if hh + 1 < H:
                emit_pairs(hh + 1)
            if hh >= 2:
                emit_norm(hh - 2)
            if hh >= 1:
                emit_recip(hh - 1, bcast=(hh - 1 < 3))
            for i, mcp in enumerate((2, 3, 0, 1)):
                emit_mms(hh, mcp, first=(i == 0), last=(i == 3))
        emit_norm(2)
        emit_recip(3, bcast=False)
        emit_norm_pss(3)

        # ---- projection + bias + residual ---------------------------------
        t_all = big.tile([P, NCH, D], F32)
        mvall = big.tile([P, NCH, 2], F32)
        for nb in range(NCH):
            psp = pss.tile([P, D], F32, tag="ps")
            for k in range(KCH):
                nc.tensor.matmul(
                    psp, lhsT=hmT[:, k, ts(nb, P)], rhs=pwt_sb[:, k, :],
                    start=(k == 0), stop=False)
            nc.tensor.matmul(psp, lhsT=ones_sb, rhs=pb_sb,
                             start=False, stop=False)
            for k in range(KCH):
                nc.tensor.matmul(psp[:, ts(k, P)],
                                 lhsT=hT_sb[:, k, ts(nb, P)], rhs=ident,
                                 start=False, stop=(k == KCH - 1))
            nc.scalar.activation(out=t_all[:, nb, :], in_=psp, func=Copy)
            stats = small.tile([P, 6], F32, tag="stats", bufs=2)
            nc.vector.bn_stats(out=stats, in_=t_all[:, nb, :])
            nc.vector.bn_aggr(out=mvall[:, nb, :], in_=stats)

        # ---- rsqrt(var+eps) via bit trick + 2 Newton steps (2 batches) ----
        var = small.tile([P, NCH], F32, tag="var")
        sh = small.tile([P, NCH], I32, tag="sh")
        yg = small.tile([P, NCH], I32, tag="yg")
        t1 = small.tile([P, NCH], F32, tag="nt1")
        t2 = small.tile([P, NCH], F32, tag="nt2")
        rsd = small.tile([P, NCH], F32, tag="rsd")
        nbias = small.tile([P, NCH], F32, tag="nbias")
        out_sb = big.tile([P, NCH, D], F32)
        yf = yg.bitcast(F32)
        for g in range(2):
            gs = slice(4 * g, 4 * g + 4)
            nc.vector.tensor_scalar(out=var[:, gs], in0=mvall[:, gs, 1],
                                    scalar1=1.0, scalar2=float(EPS),
                                    op0=mult, op1=add)
            nc.vector.tensor_scalar(out=sh[:, gs], in0=var[:, gs].bitcast(I32),
                                    scalar1=1, scalar2=None, op0=ashr)
            nc.vector.tensor_scalar(out=yg[:, gs], in0=sh[:, gs], scalar1=-1,
                                    scalar2=0x5F3759DF, op0=mult, op1=add)
            for it in range(1):
                src = yf[:, gs] if it == 0 else rsd[:, gs]
                nc.vector.tensor_tensor(out=t1[:, gs], in0=src, in1=src,
                                        op=mult)
                nc.vector.tensor_tensor(out=t2[:, gs], in0=t1[:, gs],
                                        in1=var[:, gs], op=mult)
                nc.vector.tensor_scalar(out=t2[:, gs], in0=t2[:, gs],
                                        scalar1=-0.5, scalar2=1.5,
                                        op0=mult, op1=add)
                nc.vector.tensor_tensor(out=rsd[:, gs], in0=t2[:, gs],
                                        in1=src, op=mult)
            nc.vector.tensor_tensor(out=nbias[:, gs], in0=mvall[:, gs, 0],
                                    in1=rsd[:, gs], op=mult)
            nc.vector.tensor_scalar(out=nbias[:, gs], in0=nbias[:, gs],
                                    scalar1=-1.0, scalar2=None, op0=mult)
            for nb in range(4 * g, 4 * g + 4):
                t2b = work.tile([P, D], BF16, tag="t2b")
                nc.scalar.activation(
                    out=t2b, in_=t_all[:, nb, :], func=Ident,
                    bias=nbias[:, nb:nb + 1], scale=rsd[:, nb:nb + 1])
                gb_eng = nc.gpsimd if nb in (0, 4) else nc.vector
                t3 = work.tile([P, D], F32, tag="t3")
                gb_eng.tensor_tensor(out=t3, in0=t2b, in1=gam_bc, op=mult)
                gb_eng.tensor_tensor(out=out_sb[:, nb, :], in0=t3, in1=bet_bc,
                                     op=add)
                nc.sync.dma_start(
                    out=out_d.rearrange("(c p) d -> p c d", p=P)[:, nb, :],
                    in_=out_sb[:, nb, :])


def _get_nc():
    if "nc" not in _CACHE:
        _CACHE["nc"] = _build_bass()
    return _CACHE["nc"]


def prepare_in_maps(h, adj, W, a1, a2, proj_w, proj_b, gamma, beta):
    h = np.asarray(h, np.float32)
    adj = np.asarray(adj)
    W = np.asarray(W, np.float32)
    a1 = np.asarray(a1, np.float32)
    a2 = np.asarray(a2, np.float32)
    proj_w = np.asarray(proj_w, np.float32)
    proj_b = np.asarray(proj_b, np.float32)
    gamma = np.asarray(gamma, np.float32)
    beta = np.asarray(beta, np.float32)

    bf = ml_dtypes.bfloat16
    adjT = np.ascontiguousarray(adj.T.astype(np.float32)).astype(bf)
    liq = np.where(np.ascontiguousarray(adj.T) != 0, 0, -3000).astype(np.int16)
    wcat = np.ascontiguousarray(
        W.transpose(1, 0, 2).reshape(D, H * HD)).astype(bf)
    csj = np.zeros((D, H), np.float32)
    csi = np.zeros((D, H), np.float32)
    for hh in range(H):
        csj[:, hh] = W[hh] @ a2[hh]
        csi[:, hh] = (0.8 * A16) * (W[hh] @ a1[hh])
    wcc = np.concatenate(
        [wcat.astype(np.float32), csj, csi], axis=1).astype(bf)
    pwT = np.ascontiguousarray(proj_w.T).astype(bf)
    pb = proj_b.reshape(1, D).astype(bf)
    gam = gamma.reshape(1, D).astype(np.float32)
    bet = beta.reshape(1, D).astype(np.float32)

    in_maps = []
    for b in range(B):
        in_maps.append({
            "h_b": np.ascontiguousarray(h[b]).astype(bf),
            "hT_b": np.ascontiguousarray(h[b].T).astype(bf),
            "adjT": adjT,
            "liq": liq,
            "wcc": wcc,
            "pwT": pwT,
            "pb": pb,
            "gamma": gam,
            "beta": bet,
        })
    return in_maps


def kernel(h, adj, W, a1, a2, proj_w, proj_b, gamma, beta):
    nc = _get_nc()
    in_maps = prepare_in_maps(h, adj, W, a1, a2, proj_w, proj_b, gamma, beta)
    res = run_bass_kernel_spmd(nc, in_maps, core_ids=list(range(B)))
    out = np.stack([r["out_b"] for r in res.results], axis=0)
    return out.astype(np.float32)


# revision 43
# speedup vs baseline: 1.0129x; 1.0129x over previous
"""Multi-head graph attention (GAT) kernel for 8 Trainium2 NeuronCores.

Math (per batch b, head h):
  Wh = h @ W_h                        [N, HD]
  si = Wh @ a1_h ; sj = Wh @ a2_h     [N]
  e[n, m] = leaky_relu(si[n] + sj[m], 0.2), masked where adj[n, m] == 0
  alpha = softmax(e, axis=-1); out = alpha @ Wh; concat heads; proj; +h; LN

Device identity chain (per softmax row n; factors in si[n] cancel):
  exp(leaky(y)) = exp(0.2*y + 0.8*relu(y))
               ~= exp(0.8*relu(y) + 0.2*sj[m])           (mod exp(0.2 si[n]))
Scores are built transposed E^T[m, n] (m on partitions) so tiles feed the
attention*V matmul directly, with a ones column in the stationary [Wh | 1]
producing softmax row-sums for free.

Most tiles evaluate the exp as a Schraudolph bit-trick in ONE fused
tensor_scalar (max distributes over the affine map; A = 2^7/ln2):
  bits = max(0.8*A*si[n] + (A*sj[m] + B'), (0.2*A*sj[m] + B'))   -> int16
bitcast(int16 -> bf16) is exp(0.8*relu(y) + 0.2*sj) to ~2% relative.
One pair per head instead runs exact Relu+Exp on the Activation engine to
balance engine load. Masking multiplies by the adjacency plane (bf16),
split between DVE and GPSIMD.

Sharding: batch b -> core b (B == 8 == n_cores). adj/params replicated.
"""

import os
import sys

for _p in ("/opt/trn_rl_repo", "/root/.axon_site/_ro/trn_rl_repo"):
    if os.path.isdir(_p) and _p not in sys.path:
        sys.path.insert(0, _p)

import numpy as np
import ml_dtypes

import concourse.bass as bass
import concourse.bacc as bacc
import concourse.tile as tile
import concourse.mybir as mybir
from concourse.bass import ts
from concourse.bass_utils import run_bass_kernel_spmd

B, N, D, H, HD = 8, 1024, 256, 4, 64
P = 128
NCH = N // P  # 8 chunks of the node axis
KCH = D // P  # 2 chunks of the feature axis
EPS = 1e-5

F32 = mybir.dt.float32
BF16 = mybir.dt.bfloat16
I16 = mybir.dt.int16
I32 = mybir.dt.int32

A16 = 128.0 / np.log(2.0)         # Schraudolph scale for bf16 bit layout
BOFF = 127.0 * 128.0 - 5.7        # exponent bias + error-centering offset
WCOLS = H * HD + H + H            # [Wcat | csj | csi'] columns

_CACHE = {}


def _build_bass():
    nc = bacc.Bacc("TRN2", target_bir_lowering=False, debug=False)

    h_d = nc.dram_tensor("h_b", [N, D], BF16, kind="ExternalInput").ap()
    hT_d = nc.dram_tensor("hT_b", [D, N], BF16, kind="ExternalInput").ap()
    adjt_d = nc.dram_tensor("adjT", [N, N], BF16, kind="ExternalInput").ap()
    liq_d = nc.dram_tensor("liq", [N, N], I16, kind="ExternalInput").ap()
    wcc_d = nc.dram_tensor("wcc", [D, WCOLS], BF16, kind="ExternalInput").ap()
    pwt_d = nc.dram_tensor("pwT", [D, D], BF16, kind="ExternalInput").ap()
    pb_d = nc.dram_tensor("pb", [1, D], BF16, kind="ExternalInput").ap()
    gam_d = nc.dram_tensor("gamma", [1, D], F32, kind="ExternalInput").ap()
    bet_d = nc.dram_tensor("beta", [1, D], F32, kind="ExternalInput").ap()
    out_d = nc.dram_tensor("out_b", [N, D], F32, kind="ExternalOutput").ap()
    sis_d = nc.dram_tensor("si_scr", [H, N], BF16, kind="Internal").ap()
    rrs_d = nc.dram_tensor("rr_scr", [H, N], BF16, kind="Internal").ap()

    with tile.TileContext(nc) as tc:
        _emit(nc, tc, h_d, hT_d, adjt_d, liq_d, wcc_d, pwt_d, pb_d, gam_d,
              bet_d, out_d, sis_d, rrs_d)
    nc.compile()
    return nc


def _emit(nc, tc, h_d, hT_d, adjt_d, liq_d, wcc_d, pwt_d, pb_d, gam_d,
          bet_d, out_d, sis_d, rrs_d):
    import contextlib

    add = mybir.AluOpType.add
    mult = mybir.AluOpType.mult
    amax = mybir.AluOpType.max
    ashr = mybir.AluOpType.arith_shift_right
    Copy = mybir.ActivationFunctionType.Copy
    Ident = mybir.ActivationFunctionType.Identity
    Relu = mybir.ActivationFunctionType.Relu
    Exp = mybir.ActivationFunctionType.Exp

    ctx = contextlib.ExitStack()
    with ctx:
        const = ctx.enter_context(tc.tile_pool(name="const", bufs=1))
        big = ctx.enter_context(tc.tile_pool(name="big", bufs=1))
        work = ctx.enter_context(tc.tile_pool(name="work", bufs=4))
        small = ctx.enter_context(tc.tile_pool(name="small", bufs=1))
        pss = ctx.enter_context(tc.tile_pool(name="pss", bufs=2, space="PSUM"))
        psg = ctx.enter_context(tc.tile_pool(name="psg", bufs=3, space="PSUM"))

        # tiny dummy exp: absorbs the one activation-table load at t=0
        dummy = const.tile([1, 1], BF16)
        nc.vector.memset(dummy, 0.0)
        nc.scalar.activation(out=dummy, in_=dummy, func=Exp)

        # ---- phase-0 loads: only what the si chain needs ------------------
        hT_sb = big.tile([P, KCH, N], BF16)
        hT_r = hT_d.rearrange("(k p) n -> p k n", p=P)
        for k in range(KCH):
            nc.sync.dma_start(out=hT_sb[:, k, :], in_=hT_r[:, k, :])
        wcc_sb = const.tile([P, KCH, WCOLS], BF16)
        nc.sync.dma_start(out=wcc_sb,
                          in_=wcc_d.rearrange("(k p) m -> p k m", p=P))
        adjt_sb = big.tile([P, NCH, N], BF16)
        adjt_r = adjt_d.rearrange("(c p) n -> p c n", p=P)
        liq_r = liq_d.rearrange("(c p) n -> p c n", p=P)

        # ---- si rows (pre-scaled 0.8*A16) -> DRAM -> per-head broadcast ---
        sirow = small.tile([H, N], BF16, tag="sirow")
        for s_ in range(2):
            ps_si = pss.tile([H, 512], F32, tag="ps")
            for k in range(KCH):
                nc.tensor.matmul(
                    ps_si, lhsT=wcc_sb[:, k, H * HD + H:],
                    rhs=hT_sb[:, k, ts(s_, 512)],
                    start=(k == 0), stop=(k == KCH - 1))
            nc.scalar.activation(out=sirow[:, ts(s_, 512)], in_=ps_si,
                                 func=Copy)
        nc.sync.dma_start(out=sis_d, in_=sirow)
        sibca = big.tile([P, H, N], BF16)
        nc.sync.dma_start(
            out=sibca[:, 0:1, :],
            in_=bass.AP(tensor=sis_d.tensor, offset=sis_d.offset,
                        ap=[[0, P], [N, 1], [1, N]]))
        nc.sync.dma_start(
            out=sibca[:, 1:4, :],
            in_=bass.AP(tensor=sis_d.tensor, offset=sis_d.offset + N,
                        ap=[[0, P], [N, 3], [1, N]]))

        # ---- phase-1 loads: everything else -------------------------------
        for c2 in (0, 4):
            nc.sync.dma_start(out=adjt_sb[:, c2:c2 + 2, :],
                              in_=adjt_r[:, c2:c2 + 2, :])
        pwt_sb = const.tile([P, KCH, D], BF16)
        nc.sync.dma_start(out=pwt_sb,
                          in_=pwt_d.rearrange("(k p) m -> p k m", p=P))
        pb_sb = const.tile([1, D], BF16)
        nc.sync.dma_start(out=pb_sb, in_=pb_d)
        h_sb = big.tile([P, NCH, D], BF16)
        nc.sync.dma_start(out=h_sb, in_=h_d.rearrange("(c p) d -> p c d", p=P))
        gam_bc = const.tile([P, D], F32)
        nc.sync.dma_start(
            out=gam_bc, in_=bass.AP(tensor=gam_d.tensor, offset=gam_d.offset,
                                    ap=[[0, P], [1, D]]))
        bet_bc = const.tile([P, D], F32)
        nc.sync.dma_start(
            out=bet_bc, in_=bass.AP(tensor=bet_d.tensor, offset=bet_d.offset,
                                    ap=[[0, P], [1, D]]))

        ones_sb = const.tile([1, P], BF16)
        nc.vector.memset(ones_sb, 1.0)
        ident = const.tile([P, P], BF16)
        from concourse.masks import make_identity
        make_identity(nc, ident)

        whs_sb = big.tile([P, NCH, H, HD + 1], BF16)
        nc.vector.memset(whs_sb[:, :, :, HD:HD + 1], 1.0)

        # ---- Wh + sj columns for all chunks ------------------------------
        s1c = big.tile([P, NCH, H], F32)   # A16*sj + BOFF       (bits dom)
        s2c = big.tile([P, NCH, H], F32)   # 0.2*A16*sj + BOFF   (bits dom)
        for c in range(NCH):
            ps = pss.tile([P, H * HD + H], F32, tag="ps")
            for k in range(KCH):
                nc.tensor.matmul(
                    ps, lhsT=hT_sb[:, k, ts(c, P)],
                    rhs=wcc_sb[:, k, 0:H * HD + H],
                    start=(k == 0), stop=(k == KCH - 1))
            nc.scalar.activation(
                out=whs_sb[:, c, :, 0:HD],
                in_=ps[:, 0:H * HD].rearrange("p (h d) -> p h d", h=H),
                func=Copy)
            nc.vector.tensor_scalar(
                out=s1c[:, c, :], in0=ps[:, H * HD:], scalar1=float(A16),
                scalar2=float(BOFF), op0=mult, op1=add)
            nc.vector.tensor_scalar(
                out=s2c[:, c, :], in0=ps[:, H * HD:], scalar1=float(0.2 * A16),
                scalar2=float(BOFF), op0=mult, op1=add)
        # ACT-domain columns derived from the bits-domain ones:
        #   s3c = 0.8*A16*sj = 0.8*(s1c - BOFF);  s4c = 0.2*sj = (s2c-BOFF)/A16
        s3c = big.tile([P, NCH, H], F32)
        nc.vector.tensor_scalar(
            out=s3c, in0=s1c, scalar1=0.8, scalar2=float(-0.8 * BOFF),
            op0=mult, op1=add)
        s4c = big.tile([P, NCH, H], F32)
        nc.vector.tensor_scalar(
            out=s4c, in0=s2c, scalar1=float(1.0 / A16),
            scalar2=float(-BOFF / A16), op0=mult, op1=add)

        # ---- scores + A@V --------------------------------------------------
        # Mask engines per pair: 1 -> GPSIMD (longest cover, consumed last),
        # 3 -> DMA compute-copy (multiplies adjacency in from DRAM), 0/2 ->
        # DVE. Matmuls consume [2, 3, 0, 1]. Softmax normalization runs at a
        # two-head lag: reciprocal row -> DRAM -> partition-broadcast back,
        # then a PSUM x SBUF multiply drains into hmT.
        hmT = big.tile([P, KCH, N], BF16)
        psg_of = {}
        rr_of = {}
        rrbc_of = {}
        gm_of = {}
        DMA_PAIRS = {(hh, 3) for hh in range(H)}

        act_g_of = {}

        def emit_act_pair(hh):
            gm2 = work.tile([P, 2, N], BF16, tag="gm2a", bufs=3)
            for j in range(2):
                r2 = work.tile([P, N], BF16, tag="r2")
                nc.scalar.activation(
                    out=r2, in_=sibca[:, hh, :], func=Relu,
                    bias=s3c[:, j, hh:hh + 1])
                nc.scalar.activation(
                    out=gm2[:, j, :], in_=r2, func=Exp,
                    bias=s4c[:, j, hh:hh + 1], scale=float(1.0 / A16))
            act_g_of[hh] = gm2

        def emit_gmm(hh):
            gmm = work.tile([P, 2, N], BF16, tag="gmm", bufs=2)
            nc.vector.tensor_tensor(
                out=gmm, in0=act_g_of[hh], in1=adjt_sb[:, 0:2, :], op=mult)
            gm_of[(hh, 0)] = gmm

        def emit_g2(hh, mcp):
            g2 = work.tile([P, 2, N], I16, tag="g2", bufs=8)
            for j in range(2):
                mc = 2 * mcp + j
                nc.vector.tensor_scalar(
                    out=g2[:, j, :], in0=sibca[:, hh, :],
                    scalar1=s1c[:, mc, hh:hh + 1],
                    scalar2=s2c[:, mc, hh:hh + 1], op0=add, op1=amax)
            return g2

        def emit_mask(hh, mcp, g2):
            if (hh, mcp) in DMA_PAIRS:
                # mask via DMA compute-add: bits += {0, -3000} plane (int16)
                nc.gpsimd.dma_start(
                    out=g2, in_=liq_r[:, 2 * mcp:2 * mcp + 2, :],
                    accum_op=add)
                gm_of[(hh, mcp)] = g2.bitcast(BF16)
                return
            gm2 = work.tile([P, 2, N], BF16, tag="gm2", bufs=4)
            eng = nc.gpsimd if mcp == 1 else nc.vector
            if mcp == 1:
                for j in range(2):
                    eng.tensor_tensor(
                        out=gm2[:, j, :], in0=g2[:, j, :].bitcast(BF16),
                        in1=adjt_sb[:, 2 * mcp + j, :], op=mult)
            else:
                eng.tensor_tensor(
                    out=gm2, in0=g2.bitcast(BF16),
                    in1=adjt_sb[:, 2 * mcp:2 * mcp + 2, :], op=mult)
            gm_of[(hh, mcp)] = gm2

        def emit_pairs(hh):
            emit_act_pair(hh)
            g1 = emit_g2(hh, 1)
            emit_mask(hh, 1, g1)
            g3 = emit_g2(hh, 3)
            emit_mask(hh, 3, g3)
            g2_ = emit_g2(hh, 2)
            emit_mask(hh, 2, g2_)
            emit_gmm(hh)

        def emit_mms(hh, mcp, first, last):
            pg = psg_of[hh]
            gm = gm_of[(hh, mcp)]
            for j in range(2):
                for s_ in range(2):
                    nc.tensor.matmul(
                        pg[:, ts(s_, 512)],
                        lhsT=whs_sb[:, 2 * mcp + j, hh, :],
                        rhs=gm[:, j, ts(s_, 512)],
                        start=(first and j == 0), stop=(last and j == 1))

        def emit_recip(hh, bcast=True):
            pg = psg_of[hh]
            rr1 = work.tile([1, N], BF16, tag="rr1")
            rr_of[hh] = rr1
            with nc.allow_low_precision(reason="bf16 softmax scale"):
                nc.vector.reciprocal(out=rr1, in_=pg[HD:HD + 1, :])
            if not bcast:
                return
            nc.sync.dma_start(
                out=bass.AP(tensor=rrs_d.tensor, offset=rrs_d.offset + hh * N,
                            ap=[[1, N]]),
                in_=rr1)
            rrbc = work.tile([64, N], BF16, tag="rrbc")
            rrbc_of[hh] = rrbc
            nc.sync.dma_start(
                out=rrbc,
                in_=bass.AP(tensor=rrs_d.tensor, offset=rrs_d.offset + hh * N,
                            ap=[[0, 64], [1, N]]))

        def emit_norm(hh):
            pg = psg_of[hh]
            pr = 64 * (hh % 2)
            nc.vector.tensor_tensor(
                out=hmT[pr:pr + 64, hh // 2, :], in0=pg[0:HD, :],
                in1=rrbc_of[hh], op=mult)

        def emit_norm_pss(hh):
            # DMA-free tail normalization for the last head
            pg = psg_of[hh]
            rr1 = rr_of[hh]
            hm_un = work.tile([64, N], BF16, tag="hmun")
            nc.scalar.activation(out=hm_un, in_=pg[0:HD, :], func=Copy)
            pr = 64 * (hh % 2)
            for s_ in range(2):
                psrr = pss.tile([64, 512], F32, tag="ps")
                nc.tensor.matmul(psrr, lhsT=ones_sb[0:1, 0:64],
                                 rhs=rr1[0:1, ts(s_, 512)],
                                 start=True, stop=True)
                nc.vector.tensor_tensor(
                    out=hmT[pr:pr + 64, hh // 2, ts(s_, 512)],
                    in0=hm_un[:, ts(s_, 512)], in1=psrr, op=mult)

        for hh in range(H):
            psg_of[hh] = psg.tile([HD + 1, N], F32, tag="av", name=f"pg{hh}")
        emit_pairs(0)
        for hh in range(H):
            if hh + 1 < H:
                emit_pairs(hh + 1)
            if hh >= 2:
                emit_norm(hh - 2)
            if hh >= 1:
                emit_recip(hh - 1, bcast=(hh - 1 < 3))
            for i, mcp in enumerate((2, 0, 3, 1)):
                emit_mms(hh, mcp, first=(i == 0), last=(i == 3))
        emit_norm(2)
        emit_recip(3, bcast=False)
        emit_norm_pss(3)

        # ---- projection + bias + residual ---------------------------------
        t_all = big.tile([P, NCH, D], F32)
        mvall = big.tile([P, NCH, 2], F32)
        for nb in range(NCH):
            psp = pss.tile([P, D], F32, tag="ps")
            for k in range(KCH):
                nc.tensor.matmul(
                    psp, lhsT=hmT[:, k, ts(nb, P)], rhs=pwt_sb[:, k, :],
                    start=(k == 0), stop=False)
            nc.tensor.matmul(psp, lhsT=ones_sb, rhs=pb_sb,
                             start=False, stop=False)
            for k in range(KCH):
                nc.tensor.matmul(psp[:, ts(k, P)],
                                 lhsT=hT_sb[:, k, ts(nb, P)], rhs=ident,
                                 start=False, stop=(k == KCH - 1))
            nc.scalar.activation(out=t_all[:, nb, :], in_=psp, func=Copy)
            stats = small.tile([P, 6], F32, tag="stats", bufs=2)
            nc.vector.bn_stats(out=stats, in_=t_all[:, nb, :])
            nc.vector.bn_aggr(out=mvall[:, nb, :], in_=stats)

        # ---- rsqrt(var+eps) via bit trick + 2 Newton steps (2 batches) ----
        var = small.tile([P, NCH], F32, tag="var")
        sh = small.tile([P, NCH], I32, tag="sh")
        yg = small.tile([P, NCH], I32, tag="yg")
        t1 = small.tile([P, NCH], F32, tag="nt1")
        t2 = small.tile([P, NCH], F32, tag="nt2")
        rsd = small.tile([P, NCH], F32, tag="rsd")
        nbias = small.tile([P, NCH], F32, tag="nbias")
        out_sb = big.tile([P, NCH, D], F32)
        yf = yg.bitcast(F32)
        for g in range(2):
            gs = slice(4 * g, 4 * g + 4)
            nc.vector.tensor_scalar(out=var[:, gs], in0=mvall[:, gs, 1],
                                    scalar1=1.0, scalar2=float(EPS),
                                    op0=mult, op1=add)
            nc.vector.tensor_scalar(out=sh[:, gs], in0=var[:, gs].bitcast(I32),
                                    scalar1=1, scalar2=None, op0=ashr)
            nc.vector.tensor_scalar(out=yg[:, gs], in0=sh[:, gs], scalar1=-1,
                                    scalar2=0x5F3759DF, op0=mult, op1=add)
            for it in range(1):
                src = yf[:, gs] if it == 0 else rsd[:, gs]
                nc.vector.tensor_tensor(out=t1[:, gs], in0=src, in1=src,
                                        op=mult)
                nc.vector.tensor_tensor(out=t2[:, gs], in0=t1[:, gs],
                                        in1=var[:, gs], op=mult)
                nc.vector.tensor_scalar(out=t2[:, gs], in0=t2[:, gs],
                                        scalar1=-0.5, scalar2=1.5,
                                        op0=mult, op1=add)
                nc.vector.tensor_tensor(out=rsd[:, gs], in0=t2[:, gs],
                                        in1=src, op=mult)
            nc.vector.tensor_tensor(out=nbias[:, gs], in0=mvall[:, gs, 0],
                                    in1=rsd[:, gs], op=mult)
            nc.vector.tensor_scalar(out=nbias[:, gs], in0=nbias[:, gs],
                                    scalar1=-1.0, scalar2=None, op0=mult)
            for nb in range(4 * g, 4 * g + 4):
                t2b = work.tile([P, D], BF16, tag="t2b")
                nc.scalar.activation(
                    out=t2b, in_=t_all[:, nb, :], func=Ident,
                    bias=nbias[:, nb:nb + 1], scale=rsd[:, nb:nb + 1])
                gb_eng = nc.gpsimd if nb in (0, 4) else nc.vector
                t3 = work.tile([P, D], F32, tag="t3")
                gb_eng.tensor_tensor(out=t3, in0=t2b, in1=gam_bc, op=mult)
                gb_eng.tensor_tensor(out=out_sb[:, nb, :], in0=t3, in1=bet_bc,
                                     op=add)
                nc.sync.dma_start(
                    out=out_d.rearrange("(c p) d -> p c d", p=P)[:, nb, :],
                    in_=out_sb[:, nb, :])


def _get_nc():
    if "nc" not in _CACHE:
        _CACHE["nc"] = _build_bass()
    return _CACHE["nc"]


def prepare_in_maps(h, adj, W, a1, a2, proj_w, proj_b, gamma, beta):
    h = np.asarray(h, np.float32)
    adj = np.asarray(adj)
    W = np.asarray(W, np.float32)
    a1 = np.asarray(a1, np.float32)
    a2 = np.asarray(a2, np.float32)
    proj_w = np.asarray(proj_w, np.float32)
    proj_b = np.asarray(proj_b, np.float32)
    gamma = np.asarray(gamma, np.float32)
    beta = np.asarray(beta, np.float32)

    bf = ml_dtypes.bfloat16
    adjT = np.ascontiguousarray(adj.T.astype(np.float32)).astype(bf)
    liq = np.where(np.ascontiguousarray(adj.T) != 0, 0, -3000).astype(np.int16)
    wcat = np.ascontiguousarray(
        W.transpose(1, 0, 2).reshape(D, H * HD)).astype(bf)
    csj = np.zeros((D, H), np.float32)
    csi = np.zeros((D, H), np.float32)
    for hh in range(H):
        csj[:, hh] = W[hh] @ a2[hh]
        csi[:, hh] = (0.8 * A16) * (W[hh] @ a1[hh])
    wcc = np.concatenate(
        [wcat.astype(np.float32), csj, csi], axis=1).astype(bf)
    pwT = np.ascontiguousarray(proj_w.T).astype(bf)
    pb = proj_b.reshape(1, D).astype(bf)
    gam = gamma.reshape(1, D).astype(np.float32)
    bet = beta.reshape(1, D).astype(np.float32)

    in_maps = []
    for b in range(B):
        in_maps.append({
            "h_b": np.ascontiguousarray(h[b]).astype(bf),
            "hT_b": np.ascontiguousarray(h[b].T).astype(bf),
            "adjT": adjT,
            "liq": liq,
            "wcc": wcc,
            "pwT": pwT,
            "pb": pb,
            "gamma": gam,
            "beta": bet,
        })
    return in_maps


def kernel(h, adj, W, a1, a2, proj_w, proj_b, gamma, beta):
    nc = _get_nc()
    in_maps = prepare_in_maps(h, adj, W, a1, a2, proj_w, proj_b, gamma, beta)
    res = run_bass_kernel_spmd(nc, in_maps, core_ids=list(range(B)))
    out = np.stack([r["out_b"] for r in res.results], axis=0)
    return out.astype(np.float32)
